# revision 36
# baseline (speedup 1.0000x reference)
"""Bass/Tile TRN2 kernel for BasicAttention.

att = softmax(tanh(hidden @ W_h.T + p_att_feats) @ W_alpha + mask) @ att_feats

Shapes: B=64, N=2048, H=1024, A=512. Data-parallel over batch across 8
NeuronCores (8 batches per core); weights replicated; no collectives.

Memory-bound: p_att_feats ships as fp8-e4m3 and att_feats half in
bf16 (columns c<8) / half fp8-e4m3 (c>=8) -- measured 1.35e-2 max rel
err vs the 2e-2 gate on the fixed harness inputs -- dropping the
per-core HBM read stream to ~32MB (~91us at ~358GB/s). The host does
layout/dtype prep plus the tiny wh = hidden @ W_h.T fold (0.006% of the
FLOPs). After this the four engines are balanced: DMA ~91us, PE ~92us,
DVE ~73us, ACT ~67us.

Device per core, per batch b (software-pipelined):
  pa_T stream [128a, 4ab, 2048n] fp8 (4 DMAs, subtile deps): ACT tanh
    with per-partition bias wh_T[:, b] fused -> alpha_T bf16 (no DVE
    add needed).
  scores on PE: 64 rank-128 matmuls, lhsT = alpha_T[:, ab, c::16]
    stationary (M=128 regions -> output lands directly in the n=p*16+c
    layout), rhs = W_alpha chunk [128a, 1]. Each matmul is its own
    single-instruction accumulation group (start=stop=True) writing its
    own PSUM address: any order is legal in one bank (no open zero
    region) and there is no same-address accumulate stall. ab-outer
    order so each 16-matmul pass depends on only one tanh block. The 4
    partials per column are summed on the idle DVE.
  expnorm (deferred one iteration, issued ahead of the next tanh so the
    ACT FIFO and the dependent PE sum-matmul never stall): DVE reduce +
    mask add, ACT exp (bf16 out, f32 rowsum accum), PE ones-matmul total
    sum, DVE reciprocal.
  att reduction split across engines: the DVE multiply-accumulates
    columns t=0..5 into a per-partition partial at expnorm time (inputs
    long ready), folded into the PSUM group by one ones-matmul; the PE
    handles t=6..15 (20 matmuls, F=512 lo/hi PSUM banks; fp8 rhs for
    t>=8 mixes fine with the bf16 stationary). Scale by 1/sum (DVE),
    store via gpsimd DGE (last one via HWDGE). att phases run
    2-batch-skewed and PAIRED into high-duty PE bursts (HAM warmup paid
    once per pair); batch 7's pa/tanh/scores are pulled into iter 6 and
    att(5) runs there too, so the last att pair trails the final af
    tiles by only ~6us.
"""

import numpy as np

B, N, H, A = 64, 2048, 1024, 512
NCORES = 8
BLOC = B // NCORES  # batches per core

P = 128
NT = N // P       # 16 n-columns per partition (n = p*16 + c)
AB = A // P       # 4 a-blocks
AF_SUP = 4        # att_feats columns per supertile (4 DMAs per batch)

_NC_CACHE = {}


def _build_nc():
    import concourse.bass as bass
    import concourse.mybir as mybir
    import concourse.tile as tile
    from concourse import bacc

    dt = mybir.dt
    f32, bf16 = dt.float32, dt.bfloat16
    AF = mybir.ActivationFunctionType
    OP = mybir.AluOpType

    nc = bacc.Bacc("TRN2", target_bir_lowering=False, debug=False,
                   num_devices=NCORES)

    fp8 = dt.float8e4
    paT = nc.dram_tensor("p_att_T", [BLOC, A, N], fp8, kind="ExternalInput").ap()
    afb = nc.dram_tensor("att_feats_bf", [BLOC, N // 2, H], bf16, kind="ExternalInput").ap()
    af8 = nc.dram_tensor("att_feats_f8", [BLOC, N // 2, H], fp8, kind="ExternalInput").ap()
    am = nc.dram_tensor("att_masks", [BLOC, N], f32, kind="ExternalInput").ap()
    whb = nc.dram_tensor("wh_T", [A, BLOC], f32, kind="ExternalInput").ap()
    wa4 = nc.dram_tensor("W_alpha4", [P, AB], bf16, kind="ExternalInput").ap()
    out = nc.dram_tensor("att_out", [BLOC, H], f32, kind="ExternalOutput").ap()

    with tile.TileContext(nc) as tc:
        with (
            tc.tile_pool(name="consts", bufs=1) as consts,
            tc.tile_pool(name="patt", bufs=3) as pa_pool,
            tc.tile_pool(name="alpha", bufs=2) as alpha_pool,
            tc.tile_pool(name="afp", bufs=3) as af_pool,
            tc.tile_pool(name="small", bufs=6) as small,
            tc.tile_pool(name="arow", bufs=2) as arow_pool,
            tc.tile_pool(name="accp", bufs=4) as acc_pool,
            tc.tile_pool(name="psmisc", bufs=2, space="PSUM") as psmisc,
            tc.tile_pool(name="psscore", bufs=2, space="PSUM") as psscore,
            tc.tile_pool(name="psatt", bufs=4, space="PSUM") as psatt,
        ):
            # ---------------- setup (tiny DMAs only) ----------------
            ones_col = consts.tile([P, 1], f32, tag="ones")
            nc.vector.memset(ones_col, 1.0)
            ones_bf = consts.tile([P, 1], bf16, tag="onesbf")
            nc.vector.memset(ones_bf, 1.0)

            # consts tiles; their many-tiny-descriptor DMAs are issued
            # FIRST so descriptor generation hides under the NRT preamble
            # (issuing them after the big streams measurably delays the
            # tanh bias by ~17us).
            whb_sb = consts.tile([P, AB, BLOC], f32, tag="whb")
            wa_sb = consts.tile([P, AB], bf16, tag="wa")
            masks_sb = consts.tile([P, BLOC, NT], f32, tag="masks")

            def consts_dma():
                nc.sync.dma_start(
                    out=whb_sb, in_=whb.rearrange("(ab p) b -> p ab b", p=P))
                nc.sync.dma_start(out=wa_sb, in_=wa4)
                nc.sync.dma_start(
                    out=masks_sb, in_=am.rearrange("b (p c) -> p b c", p=P))

            # ---------------- main loop (software-pipelined) ----------------
            paT_r = [paT[b, :, :].rearrange("(ab p) n -> p ab n", p=P)
                     for b in range(BLOC)]
            afb_r = [afb[b, :, :].rearrange("(p c) h -> p c h", c=NT // 2)
                     for b in range(BLOC)]
            af8_r = [af8[b, :, :].rearrange("(p c) h -> p c h", c=NT // 2)
                     for b in range(BLOC)]

            af_tiles = {}
            sps_tiles = {}

            def patt_dma(b):
                pa_t = pa_pool.tile([P, AB, N], fp8, tag="pa", name=f"pa{b}")
                for ab in range(AB):
                    nc.sync.dma_start(out=pa_t[:, ab, :],
                                      in_=paT_r[b][:, ab, :])
                # prefetch att_feats: columns c<8 in bf16 (feeds the
                # DVE partial + PE), c>=8 in fp8 (PE only; measured 1.34e-2
                # vs the 2e-2 gate with everything else)
                aftb = af_pool.tile([P, NT // 2, H], bf16, tag="afb",
                                    name=f"afb{b}")
                nc.sync.dma_start(out=aftb, in_=afb_r[b])
                aft8 = af_pool.tile([P, NT // 2, H], fp8, tag="af8",
                                    name=f"af8{b}")
                nc.sync.dma_start(out=aft8, in_=af8_r[b])
                af_tiles[b] = (aftb, aft8)
                return pa_t

            def patt_tanh(b, pa_t):
                alpha_t = alpha_pool.tile([P, AB, N], bf16, tag="alpha",
                                          name=f"alpha{b}")
                for ab in range(AB):
                    nc.scalar.activation(
                        alpha_t[:, ab, :], pa_t[:, ab, :], AF.Tanh,
                        bias=whb_sb[:, ab, b:b + 1])
                return alpha_t

            def patt_phase(b):
                return patt_tanh(b, patt_dma(b))

            def scores_mm(b, alpha_t):
                # 64 partial scores, one single-instruction accumulation
                # group each (start=stop=True): groups never stay open, so
                # any order is legal in one PSUM bank. ab-outer order means
                # each 16-matmul pass depends on only ONE tanh ab-block, so
                # the PE never waits ~2us per block like the c-outer/
                # psum-accumulated variant did. The 4 partials per column
                # are then summed on the (idle) DVE in expnorm.
                sps = psscore.tile([P, NT, AB], f32, tag="sps",
                                   name=f"sps{b}")
                for ab in range(AB):
                    for c in range(NT):
                        # stationary = alpha_T[:, ab, c::16]  (128 n's with
                        # stride 16 -> M-dim partition p of the output)
                        nc.tensor.matmul(
                            sps[:, c, ab:ab + 1],
                            lhsT=alpha_t[:, ab, c::NT],
                            rhs=wa_sb[:, ab:ab + 1],
                            start=True, stop=True)
                sps_tiles[b] = sps

            def expnorm(b):
                sps = sps_tiles.pop(b)
                scores = small.tile([P, NT], f32, tag="scores",
                                    name=f"scores{b}")
                nc.vector.tensor_reduce(out=scores, in_=sps,
                                        axis=mybir.AxisListType.X, op=OP.add)
                nc.vector.tensor_tensor(out=scores, in0=scores,
                                        in1=masks_sb[:, b, :], op=OP.add)
                expt = small.tile([P, NT], bf16, tag="expt", name=f"expt{b}")
                rowsum = small.tile([P, 1], f32, tag="rowsum",
                                    name=f"rowsum{b}")
                nc.scalar.activation(expt, scores, AF.Exp, accum_out=rowsum)

                sum_ps = psmisc.tile([1, 1], f32, tag="mm", name=f"sum_ps{b}")
                nc.tensor.matmul(sum_ps, lhsT=rowsum, rhs=ones_col,
                                 start=True, stop=True)
                inv = small.tile([1, 1], f32, tag="inv", name=f"inv{b}")
                nc.vector.reciprocal(inv, sum_ps)

                # DVE reduces af supertile 0 (landed long ago) into a
                # per-partition partial NOW, one iteration before af_phase:
                # the fold matmul there finds it ready instantly and the PE
                # att burst shrinks by a quarter.
                exptf = small.tile([P, NT], f32, tag="exptf",
                                   name=f"exptf{b}")
                nc.vector.tensor_copy(exptf, expt)
                acc = acc_pool.tile([P, H], bf16, tag="acc", name=f"acc{b}")
                aft0, _ = af_tiles[b]
                for j in range(AF_SUP + 2):
                    if j == 0:
                        nc.vector.tensor_scalar_mul(
                            acc, aft0[:, 0, :], exptf[:, 0:1])  # noqa
                    else:
                        nc.vector.scalar_tensor_tensor(
                            out=acc, in0=aft0[:, j, :],
                            scalar=exptf[:, j:j + 1], in1=acc,
                            op0=OP.mult, op1=OP.add)
                return expt, inv, acc

            def af_phase(b, expt, inv, acc, last=False):
                att_lo = psatt.tile([1, A], f32, tag="att", name=f"attlo{b}")
                att_hi = psatt.tile([1, A], f32, tag="att", name=f"atthi{b}")
                aftb, aft8 = af_tiles[b]
                for t in range(AF_SUP + 2, NT):
                    lhs = expt[:, t:t + 1]
                    rhs_t = aftb[:, t, :] if t < NT // 2 else \
                        aft8[:, t - NT // 2, :]
                    nc.tensor.matmul(att_lo, lhsT=lhs, rhs=rhs_t[0:P, 0:A],
                                     start=(t == AF_SUP + 2), stop=False)
                    nc.tensor.matmul(att_hi, lhsT=lhs, rhs=rhs_t[0:P, A:H],
                                     start=(t == AF_SUP + 2), stop=False)
                nc.tensor.matmul(att_lo, lhsT=ones_bf, rhs=acc[:, 0:A],
                                 start=False, stop=True)
                nc.tensor.matmul(att_hi, lhsT=ones_bf, rhs=acc[:, A:H],
                                 start=False, stop=True)
                del af_tiles[b]

                att_row = arow_pool.tile([1, H], f32, tag="attrow",
                                         name=f"attrow{b}")
                nc.vector.tensor_scalar_mul(att_row[:, 0:A], att_lo, inv)
                nc.vector.tensor_scalar_mul(att_row[:, A:H], att_hi, inv)
                if last:
                    nc.sync.dma_start(out=out[b:b + 1, :], in_=att_row)
                else:
                    nc.gpsimd.dma_start(out=out[b:b + 1, :], in_=att_row)

            # Schedule: expnorm(b) deferred to iter b+1 (so exp never blocks
            # behind fresh tanh on the ACT FIFO); att phases run 2-batch
            # skewed and PAIRED into long high-duty PE bursts (HAM warmup
            # paid once per pair): iters 2:(0,) 3:(1,2) 5:(3,4) 7:(5,6)
            # end:(7,). Batch 7's pa/tanh/scores are pulled forward into
            # iter 6 (pool rings have the slack) so exp(7) completes while
            # the att(5,6) pair streams and att(7) starts the moment its
            # last af tile lands, instead of waiting out the whole
            # scores(7)->exp(7) chain after the DMA stream ends. The three
            # final att bursts then chain back-to-back (HAM stays warm).
            att_sched = {2: (0,), 3: (1, 2), 5: (3, 4)}
            state = {}
            consts_dma()
            for b in range(BLOC - 2):
                # expnorm first: exp(b-1) enters the ACT FIFO ahead of
                # tanh(b) and its inputs are already ready, so neither the
                # ACT nor the dependent PE sum matmul ever stalls.
                if b >= 1:
                    state[b - 1] = expnorm(b - 1)
                alpha_t = patt_phase(b)
                for ab_ in att_sched.get(b, ()):
                    af_phase(ab_, *state.pop(ab_))
                scores_mm(b, alpha_t)
            # iter 6: att(5) runs here (its inputs are long ready) and fills
            # the PE while pa(6)/pa(7) land; batch 7's pa/tanh/scores are
            # pulled forward so exp(7) is ready before the DMA stream ends.
            state[BLOC - 3] = expnorm(BLOC - 3)
            alpha6 = patt_phase(BLOC - 2)
            alpha7 = patt_phase(BLOC - 1)
            af_phase(BLOC - 3, *state.pop(BLOC - 3))
            scores_mm(BLOC - 2, alpha6)
            scores_mm(BLOC - 1, alpha7)
            # iter 7: both exps are ready (their scores ran in iter 6, no
            # fresh tanh ahead of them on the ACT queue); att(6)+att(7)
            # chain as the final warm pair, with att(7)'s matmuls trailing
            # the last af tiles as they land.
            state[BLOC - 2] = expnorm(BLOC - 2)
            state[BLOC - 1] = expnorm(BLOC - 1)
            af_phase(BLOC - 2, *state.pop(BLOC - 2))
            af_phase(BLOC - 1, *state.pop(BLOC - 1), last=True)

    nc.compile()
    return nc


def _get_nc():
    if "nc" not in _NC_CACHE:
        _NC_CACHE["nc"] = _build_nc()
    return _NC_CACHE["nc"]


def kernel(hidden_states, att_feats, p_att_feats, att_masks, W_h, W_alpha):
    import ml_dtypes
    from concourse.bass_utils import run_bass_kernel_spmd

    nc = _get_nc()
    bf16 = ml_dtypes.bfloat16

    af32 = np.ascontiguousarray(att_feats, dtype=np.float32)
    af_r4 = af32.reshape(B, P, NT, H)                 # n = p*16 + c
    af_bf = np.ascontiguousarray(af_r4[:, :, :NT // 2]).reshape(
        B, N // 2, H).astype(bf16)
    af_f8 = np.ascontiguousarray(af_r4[:, :, NT // 2:]).reshape(
        B, N // 2, H).astype(ml_dtypes.float8_e4m3fn)
    fp8 = ml_dtypes.float8_e4m3fn
    paT16 = np.ascontiguousarray(
        np.ascontiguousarray(p_att_feats).astype(fp8).transpose(0, 2, 1))
    am32 = np.ascontiguousarray(att_masks, dtype=np.float32)      # [B,N]
    hs32 = np.ascontiguousarray(hidden_states, dtype=np.float32)
    wh32 = np.ascontiguousarray(W_h, dtype=np.float32)
    whT_all = np.ascontiguousarray(wh32 @ hs32.T)                 # [A, B] f32
    wa16 = np.ascontiguousarray(
        np.asarray(W_alpha, dtype=np.float32).reshape(AB, P).T).astype(bf16)

    in_maps = []
    for i in range(NCORES):
        s = slice(i * BLOC, (i + 1) * BLOC)
        in_maps.append({
            "p_att_T": paT16[s],
            "att_feats_bf": af_bf[s],
            "att_feats_f8": af_f8[s],
            "att_masks": am32[s],
            "wh_T": np.ascontiguousarray(whT_all[:, s]),
            "W_alpha4": wa16,
        })

    global _LAST_IN_MAPS
    _LAST_IN_MAPS = in_maps
    res = run_bass_kernel_spmd(nc, in_maps, core_ids=list(range(NCORES)))
    return np.concatenate(
        [res.results[i]["att_out"] for i in range(NCORES)], axis=0
    ).astype(np.float32)


_LAST_IN_MAPS = None


# revision 39
# speedup vs baseline: 1.0754x; 1.0754x over previous
"""Bass/Tile TRN2 kernel for BasicAttention.

att = softmax(tanh(hidden @ W_h.T + p_att_feats) @ W_alpha + mask) @ att_feats

Shapes: B=64, N=2048, H=1024, A=512. Data-parallel over batch across 8
NeuronCores (8 batches per core); weights replicated; no collectives.

Memory-bound: p_att_feats is shipped as fp8-e4m3 (empirically 7e-3 max
rel err vs the 2e-2 gate) and att_feats as bf16, dropping the per-core
HBM read stream to ~40MB (~112us at ~358GB/s). The host does
layout/dtype prep plus the tiny wh = hidden @ W_h.T fold (0.006% of the
FLOPs) so the device pipeline starts streaming immediately.

Device per core, per batch b (software-pipelined):
  pa_T stream [128a, 4ab, 2048n] fp8 (4 DMAs, subtile deps): ACT tanh
    with per-partition bias wh_T[:, b] fused -> alpha_T bf16 (no DVE
    add needed).
  scores on PE: 64 rank-128 matmuls, lhsT = alpha_T[:, ab, c::16]
    stationary (M=128 regions -> output lands directly in the n=p*16+c
    layout), rhs = W_alpha chunk [128a, 1]. Each matmul is its own
    single-instruction accumulation group (start=stop=True) writing its
    own PSUM address: any order is legal in one bank (no open zero
    region) and there is no same-address accumulate stall. ab-outer
    order so each 16-matmul pass depends on only one tanh block. The 4
    partials per column are summed on the idle DVE.
  expnorm (deferred one iteration, issued ahead of the next tanh so the
    ACT FIFO and the dependent PE sum-matmul never stall): DVE reduce +
    mask add, ACT exp (bf16 out, f32 rowsum accum), PE ones-matmul total
    sum, DVE reciprocal.
  att reduction split across engines: the DVE multiply-accumulates
    columns t=0..3 into a per-partition partial at expnorm time (inputs
    long ready), folded into the PSUM group by one ones-matmul per
    half; the PE handles t=4..15 (24 matmuls, attn col stationary
    [128,1], F=512 lo/hi PSUM banks). Scale by 1/sum (DVE), store via
    gpsimd DGE (last one via HWDGE). att phases run 2-batch-skewed and
    PAIRED into high-duty PE bursts (HAM warmup paid once per pair);
    batch 7's pa/tanh/scores are pulled into iter 6 and att(5) runs
    there too, so the final att pair trails the last af tiles by ~6us.
    (A half-fp8 att_feats variant measured ~8us faster but its error
    reached 2.2-2.4e-2 on alternative input seeds vs the 2e-2 gate --
    rejected for robustness; this config's worst observed seed is
    1.3e-2, and 7.1e-3 on the reference inputs.)
"""

import numpy as np

B, N, H, A = 64, 2048, 1024, 512
NCORES = 8
BLOC = B // NCORES  # batches per core

P = 128
NT = N // P       # 16 n-columns per partition (n = p*16 + c)
AB = A // P       # 4 a-blocks
AF_SUP = 4        # att_feats columns per supertile (4 DMAs per batch)

_NC_CACHE = {}


def _build_nc():
    import concourse.bass as bass
    import concourse.mybir as mybir
    import concourse.tile as tile
    from concourse import bacc

    dt = mybir.dt
    f32, bf16 = dt.float32, dt.bfloat16
    AF = mybir.ActivationFunctionType
    OP = mybir.AluOpType

    nc = bacc.Bacc("TRN2", target_bir_lowering=False, debug=False,
                   num_devices=NCORES)

    fp8 = dt.float8e4
    paT = nc.dram_tensor("p_att_T", [BLOC, A, N], fp8, kind="ExternalInput").ap()
    af = nc.dram_tensor("att_feats", [BLOC, N, H], bf16, kind="ExternalInput").ap()
    am = nc.dram_tensor("att_masks", [BLOC, N], f32, kind="ExternalInput").ap()
    whb = nc.dram_tensor("wh_T", [A, BLOC], f32, kind="ExternalInput").ap()
    wa4 = nc.dram_tensor("W_alpha4", [P, AB], bf16, kind="ExternalInput").ap()
    out = nc.dram_tensor("att_out", [BLOC, H], f32, kind="ExternalOutput").ap()

    with tile.TileContext(nc) as tc:
        with (
            tc.tile_pool(name="consts", bufs=1) as consts,
            tc.tile_pool(name="patt", bufs=3) as pa_pool,
            tc.tile_pool(name="alpha", bufs=2) as alpha_pool,
            tc.tile_pool(name="afp", bufs=12) as af_pool,
            tc.tile_pool(name="small", bufs=6) as small,
            tc.tile_pool(name="arow", bufs=2) as arow_pool,
            tc.tile_pool(name="accp", bufs=4) as acc_pool,
            tc.tile_pool(name="psmisc", bufs=2, space="PSUM") as psmisc,
            tc.tile_pool(name="psscore", bufs=2, space="PSUM") as psscore,
            tc.tile_pool(name="psatt", bufs=4, space="PSUM") as psatt,
        ):
            # ---------------- setup (tiny DMAs only) ----------------
            ones_col = consts.tile([P, 1], f32, tag="ones")
            nc.vector.memset(ones_col, 1.0)
            ones_bf = consts.tile([P, 1], bf16, tag="onesbf")
            nc.vector.memset(ones_bf, 1.0)

            # consts tiles; their many-tiny-descriptor DMAs are issued
            # FIRST so descriptor generation hides under the NRT preamble
            # (issuing them after the big streams measurably delays the
            # tanh bias by ~17us).
            whb_sb = consts.tile([P, AB, BLOC], f32, tag="whb")
            wa_sb = consts.tile([P, AB], bf16, tag="wa")
            masks_sb = consts.tile([P, BLOC, NT], f32, tag="masks")

            def consts_dma():
                nc.sync.dma_start(
                    out=whb_sb, in_=whb.rearrange("(ab p) b -> p ab b", p=P))
                nc.sync.dma_start(out=wa_sb, in_=wa4)

            def masks_dma():
                nc.sync.dma_start(
                    out=masks_sb, in_=am.rearrange("b (p c) -> p b c", p=P))

            # ---------------- main loop (software-pipelined) ----------------
            paT_r = [paT[b, :, :].rearrange("(ab p) n -> p ab n", p=P)
                     for b in range(BLOC)]
            af_r = [af[b, :, :].rearrange("(p c) h -> p c h", c=NT)
                    for b in range(BLOC)]

            af_tiles = {}
            sps_tiles = {}

            def patt_dma(b):
                pa_t = pa_pool.tile([P, AB, N], fp8, tag="pa", name=f"pa{b}")
                for ab in range(AB):
                    nc.sync.dma_start(out=pa_t[:, ab, :],
                                      in_=paT_r[b][:, ab, :])
                # prefetch att_feats for this batch
                tiles = []
                for st in range(NT // AF_SUP):
                    aft = af_pool.tile([P, AF_SUP, H], bf16, tag="af",
                                       name=f"af{b}_{st}")
                    nc.sync.dma_start(
                        out=aft,
                        in_=af_r[b][:, st * AF_SUP:(st + 1) * AF_SUP, :])
                    tiles.append(aft)
                af_tiles[b] = tiles
                return pa_t

            def patt_tanh(b, pa_t):
                alpha_t = alpha_pool.tile([P, AB, N], bf16, tag="alpha",
                                          name=f"alpha{b}")
                for ab in range(AB):
                    nc.scalar.activation(
                        alpha_t[:, ab, :], pa_t[:, ab, :], AF.Tanh,
                        bias=whb_sb[:, ab, b:b + 1])
                return alpha_t

            def patt_phase(b):
                return patt_tanh(b, patt_dma(b))

            def scores_mm(b, alpha_t):
                # 64 partial scores, one single-instruction accumulation
                # group each (start=stop=True): groups never stay open, so
                # any order is legal in one PSUM bank. ab-outer order means
                # each 16-matmul pass depends on only ONE tanh ab-block, so
                # the PE never waits ~2us per block like the c-outer/
                # psum-accumulated variant did. The 4 partials per column
                # are then summed on the (idle) DVE in expnorm.
                sps = psscore.tile([P, NT, AB], f32, tag="sps",
                                   name=f"sps{b}")
                for ab in range(AB):
                    for c in range(NT):
                        # stationary = alpha_T[:, ab, c::16]  (128 n's with
                        # stride 16 -> M-dim partition p of the output)
                        nc.tensor.matmul(
                            sps[:, c, ab:ab + 1],
                            lhsT=alpha_t[:, ab, c::NT],
                            rhs=wa_sb[:, ab:ab + 1],
                            start=True, stop=True)
                sps_tiles[b] = sps

            def expnorm(b):
                sps = sps_tiles.pop(b)
                scores = small.tile([P, NT], f32, tag="scores",
                                    name=f"scores{b}")
                nc.vector.tensor_reduce(out=scores, in_=sps,
                                        axis=mybir.AxisListType.X, op=OP.add)
                nc.vector.tensor_tensor(out=scores, in0=scores,
                                        in1=masks_sb[:, b, :], op=OP.add)
                expt = small.tile([P, NT], bf16, tag="expt", name=f"expt{b}")
                rowsum = small.tile([P, 1], f32, tag="rowsum",
                                    name=f"rowsum{b}")
                nc.scalar.activation(expt, scores, AF.Exp, accum_out=rowsum)

                sum_ps = psmisc.tile([1, 1], f32, tag="mm", name=f"sum_ps{b}")
                nc.tensor.matmul(sum_ps, lhsT=rowsum, rhs=ones_col,
                                 start=True, stop=True)
                inv = small.tile([1, 1], f32, tag="inv", name=f"inv{b}")
                nc.vector.reciprocal(inv, sum_ps)

                # DVE reduces af supertile 0 (landed long ago) into a
                # per-partition partial NOW, one iteration before af_phase:
                # the fold matmul there finds it ready instantly and the PE
                # att burst shrinks by a quarter.
                exptf = small.tile([P, NT], f32, tag="exptf",
                                   name=f"exptf{b}")
                nc.vector.tensor_copy(exptf, expt)
                acc = acc_pool.tile([P, H], bf16, tag="acc", name=f"acc{b}")
                aft0 = af_tiles[b][0]
                for j in range(AF_SUP):
                    if j == 0:
                        nc.vector.tensor_scalar_mul(
                            acc, aft0[:, 0, :], exptf[:, 0:1])
                    else:
                        nc.vector.scalar_tensor_tensor(
                            out=acc, in0=aft0[:, j, :],
                            scalar=exptf[:, j:j + 1], in1=acc,
                            op0=OP.mult, op1=OP.add)
                return expt, inv, acc

            def af_phase(b, expt, inv, acc, last=False):
                att_lo = psatt.tile([1, A], f32, tag="att", name=f"attlo{b}")
                att_hi = psatt.tile([1, A], f32, tag="att", name=f"atthi{b}")
                for st in range(1, NT // AF_SUP):
                    aft = af_tiles[b][st]
                    for c in range(AF_SUP):
                        t = st * AF_SUP + c
                        lhs = expt[:, t:t + 1]
                        nc.tensor.matmul(att_lo, lhsT=lhs,
                                         rhs=aft[:, c, 0:A],
                                         start=(t == AF_SUP), stop=False)
                        nc.tensor.matmul(att_hi, lhsT=lhs,
                                         rhs=aft[:, c, A:H],
                                         start=(t == AF_SUP), stop=False)
                nc.tensor.matmul(att_lo, lhsT=ones_bf, rhs=acc[:, 0:A],
                                 start=False, stop=True)
                nc.tensor.matmul(att_hi, lhsT=ones_bf, rhs=acc[:, A:H],
                                 start=False, stop=True)
                del af_tiles[b]

                att_row = arow_pool.tile([1, H], f32, tag="attrow",
                                         name=f"attrow{b}")
                nc.vector.tensor_scalar_mul(att_row[:, 0:A], att_lo, inv)
                nc.scalar.activation(att_row[:, A:H], att_hi,
                                     AF.Copy, scale=inv)
                if last:
                    nc.sync.dma_start(out=out[b:b + 1, :], in_=att_row)
                else:
                    nc.gpsimd.dma_start(out=out[b:b + 1, :], in_=att_row)

            # Schedule: expnorm(b) deferred to iter b+1 (so exp never blocks
            # behind fresh tanh on the ACT FIFO); att phases run 2-batch
            # skewed and PAIRED into long high-duty PE bursts (HAM warmup
            # paid once per pair): iters 2:(0,) 3:(1,2) 5:(3,4) 7:(5,6)
            # end:(7,). Batch 7's pa/tanh/scores are pulled forward into
            # iter 6 (pool rings have the slack) so exp(7) completes while
            # the att(5,6) pair streams and att(7) starts the moment its
            # last af tile lands, instead of waiting out the whole
            # scores(7)->exp(7) chain after the DMA stream ends. The three
            # final att bursts then chain back-to-back (HAM stays warm).
            att_sched = {2: (0,), 3: (1, 2), 5: (3, 4)}
            state = {}
            consts_dma()
            for b in range(BLOC - 2):
                # expnorm first: exp(b-1) enters the ACT FIFO ahead of
                # tanh(b) and its inputs are already ready, so neither the
                # ACT nor the dependent PE sum matmul ever stalls.
                if b >= 1:
                    state[b - 1] = expnorm(b - 1)
                alpha_t = patt_phase(b)
                if b == 0:
                    masks_dma()
                for ab_ in att_sched.get(b, ()):
                    af_phase(ab_, *state.pop(ab_))
                scores_mm(b, alpha_t)
            # iter 6: att(5) runs here (its inputs are long ready) and fills
            # the PE while pa(6)/pa(7) land; batch 7's pa/tanh/scores are
            # pulled forward so exp(7) is ready before the DMA stream ends.
            state[BLOC - 3] = expnorm(BLOC - 3)
            alpha6 = patt_phase(BLOC - 2)
            alpha7 = patt_phase(BLOC - 1)
            af_phase(BLOC - 3, *state.pop(BLOC - 3))
            scores_mm(BLOC - 2, alpha6)
            scores_mm(BLOC - 1, alpha7)
            # iter 7: both exps are ready (their scores ran in iter 6, no
            # fresh tanh ahead of them on the ACT queue); att(6)+att(7)
            # chain as the final warm pair, with att(7)'s matmuls trailing
            # the last af tiles as they land.
            state[BLOC - 2] = expnorm(BLOC - 2)
            state[BLOC - 1] = expnorm(BLOC - 1)
            af_phase(BLOC - 2, *state.pop(BLOC - 2))
            af_phase(BLOC - 1, *state.pop(BLOC - 1), last=True)

    nc.compile()
    return nc


def _get_nc():
    if "nc" not in _NC_CACHE:
        _NC_CACHE["nc"] = _build_nc()
    return _NC_CACHE["nc"]


def kernel(hidden_states, att_feats, p_att_feats, att_masks, W_h, W_alpha):
    import ml_dtypes
    from concourse.bass_utils import run_bass_kernel_spmd

    nc = _get_nc()
    bf16 = ml_dtypes.bfloat16

    af16 = np.ascontiguousarray(att_feats).astype(bf16)           # [B,N,H]
    fp8 = ml_dtypes.float8_e4m3fn
    paT16 = np.ascontiguousarray(
        np.ascontiguousarray(p_att_feats).astype(fp8).transpose(0, 2, 1))
    am32 = np.ascontiguousarray(att_masks, dtype=np.float32)      # [B,N]
    hs32 = np.ascontiguousarray(hidden_states, dtype=np.float32)
    wh32 = np.ascontiguousarray(W_h, dtype=np.float32)
    whT_all = np.ascontiguousarray(wh32 @ hs32.T)                 # [A, B] f32
    wa16 = np.ascontiguousarray(
        np.asarray(W_alpha, dtype=np.float32).reshape(AB, P).T).astype(bf16)

    in_maps = []
    for i in range(NCORES):
        s = slice(i * BLOC, (i + 1) * BLOC)
        in_maps.append({
            "p_att_T": paT16[s],
            "att_feats": af16[s],
            "att_masks": am32[s],
            "wh_T": np.ascontiguousarray(whT_all[:, s]),
            "W_alpha4": wa16,
        })

    global _LAST_IN_MAPS
    _LAST_IN_MAPS = in_maps
    res = run_bass_kernel_spmd(nc, in_maps, core_ids=list(range(NCORES)))
    return np.concatenate(
        [res.results[i]["att_out"] for i in range(NCORES)], axis=0
    ).astype(np.float32)


_LAST_IN_MAPS = None


# revision 40
# speedup vs baseline: 1.1120x; 1.0340x over previous
"""Bass/Tile TRN2 kernel for BasicAttention.

att = softmax(tanh(hidden @ W_h.T + p_att_feats) @ W_alpha + mask) @ att_feats

Shapes: B=64, N=2048, H=1024, A=512. Data-parallel over batch across 8
NeuronCores (8 batches per core); weights replicated; no collectives.

Memory-bound: p_att_feats is shipped as fp8-e4m3 (empirically 7e-3 max
rel err vs the 2e-2 gate) and att_feats as bf16, dropping the per-core
HBM read stream to ~40MB (~112us at ~358GB/s). The host does
layout/dtype prep plus the tiny wh = hidden @ W_h.T fold (0.006% of the
FLOPs) so the device pipeline starts streaming immediately.

Device per core, per batch b (software-pipelined):
  pa_T stream [128a, 4ab, 2048n] fp8 (4 DMAs, subtile deps): ACT tanh
    with per-partition bias wh_T[:, b] fused -> alpha_T bf16 (no DVE
    add needed).
  scores on PE: 64 rank-128 matmuls, lhsT = alpha_T[:, ab, c::16]
    stationary (M=128 regions -> output lands directly in the n=p*16+c
    layout), rhs = W_alpha chunk [128a, 1]. Each matmul is its own
    single-instruction accumulation group (start=stop=True) writing its
    own PSUM address: any order is legal in one bank (no open zero
    region) and there is no same-address accumulate stall. ab-outer
    order so each 16-matmul pass depends on only one tanh block. The 4
    partials per column are summed on the idle DVE.
  expnorm (deferred one iteration, issued ahead of the next tanh so the
    ACT FIFO and the dependent PE sum-matmul never stall): DVE reduce +
    mask add, ACT exp (bf16 out, f32 rowsum accum), PE ones-matmul total
    sum, DVE reciprocal.
  att reduction split across engines: the DVE multiply-accumulates
    columns t=0..3 into a per-partition partial at expnorm time (inputs
    long ready), folded into the PSUM group by one ones-matmul per
    half; the PE handles t=4..15 (24 matmuls, attn col stationary
    [128,1], F=512 lo/hi PSUM banks). Scale by 1/sum (DVE), store via
    gpsimd DGE (last one via HWDGE). att phases run 2-batch-skewed and
    PAIRED into high-duty PE bursts (HAM warmup paid once per pair);
    batch 7's pa/tanh/scores are pulled into iter 6 and att(5) runs
    there too, so the final att pair trails the last af tiles by ~6us.
    (A half-fp8 att_feats variant measured ~8us faster but its error
    reached 2.2-2.4e-2 on alternative input seeds vs the 2e-2 gate --
    rejected for robustness; this config's worst observed seed is
    1.3e-2, and 7.1e-3 on the reference inputs.)
"""

import numpy as np

B, N, H, A = 64, 2048, 1024, 512
NCORES = 8
BLOC = B // NCORES  # batches per core

P = 128
NT = N // P       # 16 n-columns per partition (n = p*16 + c)
AB = A // P       # 4 a-blocks
AF_SUP = 4        # att_feats columns per supertile (4 DMAs per batch)

_NC_CACHE = {}


def _build_nc():
    import concourse.bass as bass
    import concourse.mybir as mybir
    import concourse.tile as tile
    from concourse import bacc

    dt = mybir.dt
    f32, bf16 = dt.float32, dt.bfloat16
    AF = mybir.ActivationFunctionType
    OP = mybir.AluOpType

    nc = bacc.Bacc("TRN2", target_bir_lowering=False, debug=False,
                   num_devices=NCORES)

    fp8 = dt.float8e4
    paT = nc.dram_tensor("p_att_T", [BLOC, A, N], fp8, kind="ExternalInput").ap()
    af = nc.dram_tensor("att_feats", [BLOC, N, H], bf16, kind="ExternalInput").ap()
    am = nc.dram_tensor("att_masks", [BLOC, N], f32, kind="ExternalInput").ap()
    whb = nc.dram_tensor("wh_T", [A, BLOC], f32, kind="ExternalInput").ap()
    wa4 = nc.dram_tensor("W_alpha4", [P, AB], bf16, kind="ExternalInput").ap()
    out = nc.dram_tensor("att_out", [BLOC, H], f32, kind="ExternalOutput").ap()

    with tile.TileContext(nc) as tc:
        with (
            tc.tile_pool(name="consts", bufs=1) as consts,
            tc.tile_pool(name="patt", bufs=3) as pa_pool,
            tc.tile_pool(name="alpha", bufs=2) as alpha_pool,
            tc.tile_pool(name="afp", bufs=12) as af_pool,
            tc.tile_pool(name="small", bufs=6) as small,
            tc.tile_pool(name="arow", bufs=2) as arow_pool,
            tc.tile_pool(name="accp", bufs=4) as acc_pool,
            tc.tile_pool(name="psmisc", bufs=2, space="PSUM") as psmisc,
            tc.tile_pool(name="psscore", bufs=2, space="PSUM") as psscore,
            tc.tile_pool(name="psatt", bufs=4, space="PSUM") as psatt,
        ):
            # ---------------- setup (tiny DMAs only) ----------------
            ones_col = consts.tile([P, 1], f32, tag="ones")
            nc.vector.memset(ones_col, 1.0)
            ones_bf = consts.tile([P, 1], bf16, tag="onesbf")
            nc.vector.memset(ones_bf, 1.0)

            # consts tiles; their many-tiny-descriptor DMAs are issued
            # FIRST so descriptor generation hides under the NRT preamble
            # (issuing them after the big streams measurably delays the
            # tanh bias by ~17us).
            whb_sb = consts.tile([P, AB, BLOC], f32, tag="whb")
            wa_sb = consts.tile([P, AB], bf16, tag="wa")
            masks_sb = consts.tile([P, BLOC, NT], f32, tag="masks")

            def consts_dma():
                nc.sync.dma_start(
                    out=whb_sb, in_=whb.rearrange("(ab p) b -> p ab b", p=P))
                nc.sync.dma_start(out=wa_sb, in_=wa4)
                nc.sync.dma_start(
                    out=masks_sb, in_=am.rearrange("b (p c) -> p b c", p=P))

            # ---------------- main loop (software-pipelined) ----------------
            paT_r = [paT[b, :, :].rearrange("(ab p) n -> p ab n", p=P)
                     for b in range(BLOC)]
            af_r = [af[b, :, :].rearrange("(p c) h -> p c h", c=NT)
                    for b in range(BLOC)]

            af_tiles = {}
            sps_tiles = {}

            def patt_dma(b):
                pa_t = pa_pool.tile([P, AB, N], fp8, tag="pa", name=f"pa{b}")
                for ab in range(AB):
                    nc.sync.dma_start(out=pa_t[:, ab, :],
                                      in_=paT_r[b][:, ab, :])
                # prefetch att_feats for this batch
                tiles = []
                for st in range(NT // AF_SUP):
                    aft = af_pool.tile([P, AF_SUP, H], bf16, tag="af",
                                       name=f"af{b}_{st}")
                    nc.sync.dma_start(
                        out=aft,
                        in_=af_r[b][:, st * AF_SUP:(st + 1) * AF_SUP, :])
                    tiles.append(aft)
                af_tiles[b] = tiles
                return pa_t

            def patt_tanh(b, pa_t):
                alpha_t = alpha_pool.tile([P, AB, N], bf16, tag="alpha",
                                          name=f"alpha{b}")
                for ab in range(AB):
                    nc.scalar.activation(
                        alpha_t[:, ab, :], pa_t[:, ab, :], AF.Tanh,
                        bias=whb_sb[:, ab, b:b + 1])
                return alpha_t

            def patt_phase(b):
                return patt_tanh(b, patt_dma(b))

            def scores_mm(b, alpha_t):
                # 64 partial scores, one single-instruction accumulation
                # group each (start=stop=True): groups never stay open, so
                # any order is legal in one PSUM bank. ab-outer order means
                # each 16-matmul pass depends on only ONE tanh ab-block, so
                # the PE never waits ~2us per block like the c-outer/
                # psum-accumulated variant did. The 4 partials per column
                # are then summed on the (idle) DVE in expnorm.
                sps = psscore.tile([P, NT, AB], f32, tag="sps",
                                   name=f"sps{b}")
                for ab in range(AB):
                    for c in range(NT):
                        # stationary = alpha_T[:, ab, c::16]  (128 n's with
                        # stride 16 -> M-dim partition p of the output)
                        nc.tensor.matmul(
                            sps[:, c, ab:ab + 1],
                            lhsT=alpha_t[:, ab, c::NT],
                            rhs=wa_sb[:, ab:ab + 1],
                            start=True, stop=True)
                sps_tiles[b] = sps

            def expnorm(b):
                sps = sps_tiles.pop(b)
                scores = small.tile([P, NT], f32, tag="scores",
                                    name=f"scores{b}")
                nc.vector.tensor_reduce(out=scores, in_=sps,
                                        axis=mybir.AxisListType.X, op=OP.add)
                nc.vector.tensor_tensor(out=scores, in0=scores,
                                        in1=masks_sb[:, b, :], op=OP.add)
                expt = small.tile([P, NT], bf16, tag="expt", name=f"expt{b}")
                rowsum = small.tile([P, 1], f32, tag="rowsum",
                                    name=f"rowsum{b}")
                nc.scalar.activation(expt, scores, AF.Exp, accum_out=rowsum)

                sum_ps = psmisc.tile([1, 1], f32, tag="mm", name=f"sum_ps{b}")
                nc.tensor.matmul(sum_ps, lhsT=rowsum, rhs=ones_col,
                                 start=True, stop=True)
                inv = small.tile([1, 1], f32, tag="inv", name=f"inv{b}")
                nc.vector.reciprocal(inv, sum_ps)

                # DVE reduces af supertile 0 (landed long ago) into a
                # per-partition partial NOW, one iteration before af_phase:
                # the fold matmul there finds it ready instantly and the PE
                # att burst shrinks by a quarter.
                exptf = small.tile([P, NT], f32, tag="exptf",
                                   name=f"exptf{b}")
                nc.vector.tensor_copy(exptf, expt)
                acc = acc_pool.tile([P, H], bf16, tag="acc", name=f"acc{b}")
                aft0 = af_tiles[b][0]
                for j in range(AF_SUP):
                    if j == 0:
                        nc.vector.tensor_scalar_mul(
                            acc, aft0[:, 0, :], exptf[:, 0:1])
                    else:
                        nc.vector.scalar_tensor_tensor(
                            out=acc, in0=aft0[:, j, :],
                            scalar=exptf[:, j:j + 1], in1=acc,
                            op0=OP.mult, op1=OP.add)
                return expt, inv, acc

            def af_phase(b, expt, inv, acc, last=False):
                att_lo = psatt.tile([1, A], f32, tag="att", name=f"attlo{b}")
                att_hi = psatt.tile([1, A], f32, tag="att", name=f"atthi{b}")
                for st in range(1, NT // AF_SUP):
                    aft = af_tiles[b][st]
                    for c in range(AF_SUP):
                        t = st * AF_SUP + c
                        lhs = expt[:, t:t + 1]
                        nc.tensor.matmul(att_lo, lhsT=lhs,
                                         rhs=aft[:, c, 0:A],
                                         start=(t == AF_SUP), stop=False)
                        nc.tensor.matmul(att_hi, lhsT=lhs,
                                         rhs=aft[:, c, A:H],
                                         start=(t == AF_SUP), stop=False)
                nc.tensor.matmul(att_lo, lhsT=ones_bf, rhs=acc[:, 0:A],
                                 start=False, stop=True)
                nc.tensor.matmul(att_hi, lhsT=ones_bf, rhs=acc[:, A:H],
                                 start=False, stop=True)
                del af_tiles[b]

                att_row = arow_pool.tile([1, H], f32, tag="attrow",
                                         name=f"attrow{b}")
                nc.vector.tensor_scalar_mul(att_row[:, 0:A], att_lo, inv)
                nc.vector.tensor_scalar_mul(att_row[:, A:H], att_hi, inv)
                if last:
                    nc.sync.dma_start(out=out[b:b + 1, :], in_=att_row)
                else:
                    nc.gpsimd.dma_start(out=out[b:b + 1, :], in_=att_row)

            # Schedule: expnorm(b) deferred to iter b+1 (so exp never blocks
            # behind fresh tanh on the ACT FIFO); att phases run 2-batch
            # skewed and PAIRED into long high-duty PE bursts (HAM warmup
            # paid once per pair): iters 2:(0,) 3:(1,2) 5:(3,4) 7:(5,6)
            # end:(7,). Batch 7's pa/tanh/scores are pulled forward into
            # iter 6 (pool rings have the slack) so exp(7) completes while
            # the att(5,6) pair streams and att(7) starts the moment its
            # last af tile lands, instead of waiting out the whole
            # scores(7)->exp(7) chain after the DMA stream ends. The three
            # final att bursts then chain back-to-back (HAM stays warm).
            att_sched = {2: (0,), 3: (1, 2), 5: (3, 4)}
            state = {}
            consts_dma()
            for b in range(BLOC - 2):
                # expnorm first: exp(b-1) enters the ACT FIFO ahead of
                # tanh(b) and its inputs are already ready, so neither the
                # ACT nor the dependent PE sum matmul ever stalls.
                if b >= 1:
                    state[b - 1] = expnorm(b - 1)
                alpha_t = patt_phase(b)
                for ab_ in att_sched.get(b, ()):
                    af_phase(ab_, *state.pop(ab_))
                scores_mm(b, alpha_t)
            # iter 6: att(5) runs here (its inputs are long ready) and fills
            # the PE while pa(6)/pa(7) land; batch 7's pa/tanh/scores are
            # pulled forward so exp(7) is ready before the DMA stream ends.
            state[BLOC - 3] = expnorm(BLOC - 3)
            alpha6 = patt_phase(BLOC - 2)
            alpha7 = patt_phase(BLOC - 1)
            af_phase(BLOC - 3, *state.pop(BLOC - 3))
            scores_mm(BLOC - 2, alpha6)
            scores_mm(BLOC - 1, alpha7)
            # iter 7: both exps are ready (their scores ran in iter 6, no
            # fresh tanh ahead of them on the ACT queue); att(6)+att(7)
            # chain as the final warm pair, with att(7)'s matmuls trailing
            # the last af tiles as they land.
            state[BLOC - 2] = expnorm(BLOC - 2)
            state[BLOC - 1] = expnorm(BLOC - 1)
            af_phase(BLOC - 2, *state.pop(BLOC - 2))
            af_phase(BLOC - 1, *state.pop(BLOC - 1), last=True)

    nc.compile()
    return nc


def _get_nc():
    if "nc" not in _NC_CACHE:
        _NC_CACHE["nc"] = _build_nc()
    return _NC_CACHE["nc"]


def kernel(hidden_states, att_feats, p_att_feats, att_masks, W_h, W_alpha):
    import ml_dtypes
    from concourse.bass_utils import run_bass_kernel_spmd

    nc = _get_nc()
    bf16 = ml_dtypes.bfloat16

    af16 = np.ascontiguousarray(att_feats).astype(bf16)           # [B,N,H]
    fp8 = ml_dtypes.float8_e4m3fn
    paT16 = np.ascontiguousarray(
        np.ascontiguousarray(p_att_feats).astype(fp8).transpose(0, 2, 1))
    am32 = np.ascontiguousarray(att_masks, dtype=np.float32)      # [B,N]
    hs32 = np.ascontiguousarray(hidden_states, dtype=np.float32)
    wh32 = np.ascontiguousarray(W_h, dtype=np.float32)
    whT_all = np.ascontiguousarray(wh32 @ hs32.T)                 # [A, B] f32
    wa16 = np.ascontiguousarray(
        np.asarray(W_alpha, dtype=np.float32).reshape(AB, P).T).astype(bf16)

    in_maps = []
    for i in range(NCORES):
        s = slice(i * BLOC, (i + 1) * BLOC)
        in_maps.append({
            "p_att_T": paT16[s],
            "att_feats": af16[s],
            "att_masks": am32[s],
            "wh_T": np.ascontiguousarray(whT_all[:, s]),
            "W_alpha4": wa16,
        })

    global _LAST_IN_MAPS
    _LAST_IN_MAPS = in_maps
    res = run_bass_kernel_spmd(nc, in_maps, core_ids=list(range(NCORES)))
    return np.concatenate(
        [res.results[i]["att_out"] for i in range(NCORES)], axis=0
    ).astype(np.float32)


_LAST_IN_MAPS = None
